# revision 1
# baseline (speedup 1.0000x reference)
"""Trainium2 Bass kernel for nn_LogicalOperatorFusion.

Semantics (matches the jax reference):
  fv = field_vectors                                  [B=1024, NF=64, H=512] f32
  fv[:, not_idx] = tanh(fv[:, not_idx] @ not_W + not_b)
  cat_and = concat(fv[:, and_pairs[:,0]], fv[:, and_pairs[:,1]], -1)   [B,16,1024]
  fused_and = relu(cat_and @ and_W1 + and_b1) @ and_W2 + and_b2        [B,16,512]
  cat_or  = ... same with or_pairs (8 pairs) ...                       [B,8,512]
  out = concat([fused_and, fused_or, fv[:, leftover_idx]], axis=1)     [B,40,512]

Strategy:
  - Data parallel: batch sharded 8 ways (128 rows/core), weights replicated.
  - Host pre-transposes activations so the PE stationary operand (lhsT =
    X^T chunks, [K=128 h, M=128 rows]) arrives DMA-ready; weights are the
    N=512 moving operand.  Only the hidden layer h needs on-chip transposes
    (4 PE identity-transposes per pair).
  - Biases are applied as rank-1 (K=1) ones (x) b matmuls accumulated in PSUM,
    so ACT does single-pass relu/tanh PSUM->SBUF.
  - Untouched leftover fields (not in not_idx) never touch the device; the
    host copies them straight from the input.
"""

import sys

for p in ("/opt/trn_rl_repo",):
    if p not in sys.path:
        sys.path.insert(0, p)

import numpy as np
import ml_dtypes

BF16_NP = ml_dtypes.bfloat16

import concourse.bacc as bacc
import concourse.mybir as mybir
from concourse.bass_utils import run_bass_kernel_spmd
from concourse.tile import TileContext

B, NF, H = 1024, 64, 512
NCORES = 8
BS = B // NCORES  # 128 rows per core
NAND, NOR, NNOT = 16, 8, 8
NPAIR = NAND + NOR  # 24
NOUT = NPAIR + NNOT  # 32 device output slots
KC1 = (2 * H) // 128  # 8 k-chunks for layer 1
KC2 = H // 128  # 4 k-chunks for layer 2 / not
F32 = mybir.dt.float32
BF16 = mybir.dt.bfloat16

TRACE = False  # test.py sets this for profiled runs
LAST_RESULT = None  # BassKernelResults of the last run (for test.py)

_NC = None  # cached traced Bass program


def _build_nc():
    """Trace the per-core Bass program (identical on all 8 cores)."""
    nc = bacc.Bacc("TRN2", target_bir_lowering=False)

    pairs_in = nc.dram_tensor("pairs_in", [NPAIR, 128, 2 * H], BF16, kind="ExternalInput")
    nots_in = nc.dram_tensor("nots_in", [NNOT, 128, H], BF16, kind="ExternalInput")
    and_w1t = nc.dram_tensor("and_w1t", [128, KC1 * H], BF16, kind="ExternalInput")
    or_w1t = nc.dram_tensor("or_w1t", [128, KC1 * H], BF16, kind="ExternalInput")
    and_w2t = nc.dram_tensor("and_w2t", [128, KC2 * H], BF16, kind="ExternalInput")
    or_w2t = nc.dram_tensor("or_w2t", [128, KC2 * H], BF16, kind="ExternalInput")
    not_wt = nc.dram_tensor("not_wt", [128, KC2 * H], BF16, kind="ExternalInput")
    biases_in = nc.dram_tensor("biases", [5, 1, H], BF16, kind="ExternalInput")
    b2bc_in = nc.dram_tensor("b2bc", [2, 128, H], F32, kind="ExternalInput")
    b1bc_in = nc.dram_tensor("b1bc", [2, 128, H], F32, kind="ExternalInput")
    ident_in = nc.dram_tensor("ident", [128, 128], BF16, kind="ExternalInput")
    out_d = nc.dram_tensor("out", [NOUT, 128, H], F32, kind="ExternalOutput")

    with TileContext(nc) as tc:
        with (
            tc.tile_pool(name="consts", bufs=1) as consts,
            tc.tile_pool(name="weights", bufs=1) as wpool,
            tc.tile_pool(name="pairin", bufs=4) as inpool,
            tc.tile_pool(name="notin", bufs=3) as notpool,
            tc.tile_pool(name="hid", bufs=3) as hpool,
            tc.tile_pool(name="hidT", bufs=3) as htpool,
            tc.tile_pool(name="outs", bufs=4) as opool,
            tc.tile_pool(name="psum", bufs=2, space="PSUM") as psum,
        ):
            ident = consts.tile([128, 128], BF16, tag="ident")
            nc.sync.dma_start(out=ident, in_=ident_in[:, :])
            ones = consts.tile([1, 128], BF16, tag="ones")
            nc.vector.memset(ones, 1.0)
            bt = []
            for i in range(5):  # and_b1, or_b1, and_b2, or_b2, not_b
                b = consts.tile([1, H], BF16, tag=f"bias{i}")
                nc.sync.dma_start(out=b, in_=biases_in[i, :, :])
                bt.append(b)
            b_not = bt[4]
            b2bc_and = consts.tile([128, H], F32, tag="b2bca")
            nc.sync.dma_start(out=b2bc_and, in_=b2bc_in[0, :, :])
            b2bc_or = consts.tile([128, H], F32, tag="b2bco")
            nc.sync.dma_start(out=b2bc_or, in_=b2bc_in[1, :, :])
            b2bc = {True: b2bc_and, False: b2bc_or}
            b1bc_and = consts.tile([128, H], F32, tag="b1bca")
            nc.sync.dma_start(out=b1bc_and, in_=b1bc_in[0, :, :])
            b1bc_or = consts.tile([128, H], F32, tag="b1bco")
            nc.sync.dma_start(out=b1bc_or, in_=b1bc_in[1, :, :])
            b1bc = {True: b1bc_and, False: b1bc_or}

            w1_and = wpool.tile([128, KC1 * H], BF16, tag="w1a")
            nc.sync.dma_start(out=w1_and, in_=and_w1t[:, :])
            w2_and = wpool.tile([128, KC2 * H], BF16, tag="w2a")
            nc.sync.dma_start(out=w2_and, in_=and_w2t[:, :])
            w1_or = wpool.tile([128, KC1 * H], BF16, tag="w1o")
            w2_or = wpool.tile([128, KC2 * H], BF16, tag="w2o")
            w_not = wpool.tile([128, KC2 * H], BF16, tag="wn")
            w1 = {True: w1_and, False: w1_or}
            w2 = {True: w2_and, False: w2_or}

            hstate = {}

            def emit_l1(j):
                is_and = j < NAND
                t = inpool.tile([128, 2 * H], BF16, tag="pt_in")
                nc.sync.dma_start(out=t, in_=pairs_in[j, :, :])
                ph = psum.tile([128, H], F32, tag="ps_h")
                for k in range(KC1):
                    nc.tensor.matmul(
                        ph,
                        t[:, k * 128 : (k + 1) * 128],
                        w1[is_and][:, k * H : (k + 1) * H],
                        start=(k == 0),
                        stop=(k == KC1 - 1),
                    )
                htmp = hpool.tile([128, H], F32, tag="htmp_sb")
                nc.vector.scalar_tensor_tensor(
                    out=htmp,
                    in0=ph,
                    scalar=0.0,
                    in1=b1bc[is_and],
                    op0=mybir.AluOpType.bypass,
                    op1=mybir.AluOpType.add,
                )
                h = hpool.tile([128, H], BF16, tag="h_sb")
                nc.scalar.activation(h, htmp, mybir.ActivationFunctionType.Relu)
                hstate[j] = h

            htstate = {}

            def emit_trans(j):
                h = hstate.pop(j)
                pt = psum.tile([128, H], BF16, tag="ps_t")
                for c in range(KC2):
                    nc.tensor.transpose(
                        pt[:, c * 128 : (c + 1) * 128],
                        h[:, c * 128 : (c + 1) * 128],
                        ident,
                    )
                ht = htpool.tile([128, H], BF16, tag="ht_sb")
                nc.vector.tensor_copy(out=ht, in_=pt)
                htstate[j] = ht

            def emit_l2(j):
                is_and = j < NAND
                ht = htstate.pop(j)
                po = psum.tile([128, H], F32, tag="ps_o")
                for c in range(KC2):
                    nc.tensor.matmul(
                        po,
                        ht[:, c * 128 : (c + 1) * 128],
                        w2[is_and][:, c * H : (c + 1) * H],
                        start=(c == 0),
                        stop=(c == KC2 - 1),
                    )
                ot = opool.tile([128, H], F32, tag="o_sb")
                nc.vector.scalar_tensor_tensor(
                    out=ot,
                    in0=po,
                    scalar=0.0,
                    in1=b2bc[is_and],
                    op0=mybir.AluOpType.bypass,
                    op1=mybir.AluOpType.add,
                )
                nc.sync.dma_start(out=out_d[j, :, :], in_=ot)

            def emit_not(j):
                t = notpool.tile([128, H], BF16, tag="nt_in")
                nc.sync.dma_start(out=t, in_=nots_in[j, :, :])
                pn = psum.tile([128, H], F32, tag="ps_n")
                nc.tensor.matmul(pn, ones, b_not, start=True, stop=False)
                for c in range(KC2):
                    nc.tensor.matmul(
                        pn,
                        t[:, c * 128 : (c + 1) * 128],
                        w_not[:, c * H : (c + 1) * H],
                        start=False,
                        stop=(c == KC2 - 1),
                    )
                ot = opool.tile([128, H], F32, tag="o_sb")
                nc.scalar.activation(ot, pn, mybir.ActivationFunctionType.Tanh)
                nc.sync.dma_start(out=out_d[NPAIR + j, :, :], in_=ot)

            # 2-stage software skew: between the transposes of job j-1 and
            # their dependent L2 matmuls (which wait on the DVE copy of hT),
            # the PE runs L1(j) and L2(j-2) — no PE stall on ACT or DVE.
            emit_l1(0)
            emit_l1(1)
            emit_trans(0)
            for j in range(2, NPAIR):
                if j == 8:  # or-weights needed from job 16; queue their DMAs now
                    nc.sync.dma_start(out=w1_or, in_=or_w1t[:, :])
                if j == 12:
                    nc.sync.dma_start(out=w2_or, in_=or_w2t[:, :])
                    nc.sync.dma_start(out=w_not, in_=not_wt[:, :])
                emit_l1(j)
                emit_trans(j - 1)
                emit_l2(j - 2)
            emit_trans(NPAIR - 1)
            emit_l2(NPAIR - 2)
            emit_l2(NPAIR - 1)
            for j in range(NNOT):
                emit_not(j)

    nc.compile()
    return nc


def _get_nc():
    global _NC
    if _NC is None:
        _NC = _build_nc()
    return _NC


def _w1_tiles(W):  # [2H, H] -> [128, KC1*H], tile[p, k*H+n] = W[k*128+p, n]
    return np.ascontiguousarray(
        W.reshape(KC1, 128, H).transpose(1, 0, 2).reshape(128, KC1 * H).astype(BF16_NP)
    )


def _w2_tiles(W):  # [H, H] -> [128, KC2*H]
    return np.ascontiguousarray(
        W.reshape(KC2, 128, H).transpose(1, 0, 2).reshape(128, KC2 * H).astype(BF16_NP)
    )


def kernel(
    field_vectors,
    and_W1,
    and_b1,
    and_W2,
    and_b2,
    or_W1,
    or_b1,
    or_W2,
    or_b2,
    not_W,
    not_b,
    not_idx,
    and_pairs,
    or_pairs,
    leftover_idx,
):
    global LAST_RESULT
    fv = np.asarray(field_vectors, dtype=np.float32)
    and_W1 = np.asarray(and_W1, dtype=np.float32)
    and_W2 = np.asarray(and_W2, dtype=np.float32)
    or_W1 = np.asarray(or_W1, dtype=np.float32)
    or_W2 = np.asarray(or_W2, dtype=np.float32)
    not_W = np.asarray(not_W, dtype=np.float32)
    biases = np.stack(
        [
            np.asarray(and_b1, np.float32),
            np.asarray(or_b1, np.float32),
            np.asarray(and_b2, np.float32),
            np.asarray(or_b2, np.float32),
            np.asarray(not_b, np.float32),
        ]
    ).reshape(5, 1, H)
    not_idx = np.asarray(not_idx).astype(np.int64).ravel()
    and_pairs = np.asarray(and_pairs).astype(np.int64).reshape(NAND, 2)
    or_pairs = np.asarray(or_pairs).astype(np.int64).reshape(NOR, 2)
    leftover_idx = np.asarray(leftover_idx).astype(np.int64).ravel()

    not_set = set(not_idx.tolist())
    pair_fields = np.concatenate([and_pairs.ravel(), or_pairs.ravel()])  # 48 fields
    # The reference applies the not-transform before gathering pairs; with the
    # disjoint index structure used by this problem, pair fields are untouched.
    assert not (set(pair_fields.tolist()) & not_set), (
        "pair fields overlapping not_idx not supported"
    )

    shared = {
        "and_w1t": _w1_tiles(and_W1),
        "or_w1t": _w1_tiles(or_W1),
        "and_w2t": _w2_tiles(and_W2),
        "or_w2t": _w2_tiles(or_W2),
        "not_wt": _w2_tiles(not_W),
        "biases": np.ascontiguousarray(biases.astype(BF16_NP)),
        "b1bc": np.ascontiguousarray(
            np.stack(
                [
                    np.broadcast_to(np.asarray(and_b1, np.float32), (128, H)),
                    np.broadcast_to(np.asarray(or_b1, np.float32), (128, H)),
                ]
            )
        ),
        "b2bc": np.ascontiguousarray(
            np.stack(
                [
                    np.broadcast_to(np.asarray(and_b2, np.float32), (128, H)),
                    np.broadcast_to(np.asarray(or_b2, np.float32), (128, H)),
                ]
            )
        ),
        "ident": np.eye(128, dtype=np.float32).astype(BF16_NP),
    }

    in_maps = []
    for c in range(NCORES):
        fv_c = fv[c * BS : (c + 1) * BS]  # [128, 64, 512]
        G = fv_c[:, pair_fields, :]  # [128, 48, 512]
        G = (
            G.reshape(BS, NPAIR, 2, KC2, 128)
            .transpose(1, 4, 2, 3, 0)
            .reshape(NPAIR, 128, 2 * H)
        )
        N = fv_c[:, not_idx, :]  # [128, 8, 512]
        N = (
            N.reshape(BS, NNOT, KC2, 128)
            .transpose(1, 3, 2, 0)
            .reshape(NNOT, 128, H)
        )
        in_maps.append(
            {
                "pairs_in": np.ascontiguousarray(G.astype(BF16_NP)),
                "nots_in": np.ascontiguousarray(N.astype(BF16_NP)),
                **shared,
            }
        )

    nc = _get_nc()
    res = run_bass_kernel_spmd(nc, in_maps, core_ids=list(range(NCORES)), trace=TRACE)
    LAST_RESULT = res
    results = res.results if hasattr(res, "results") else res

    out = np.empty((B, NAND + NOR + len(leftover_idx), H), dtype=np.float32)
    not_slot = {int(f): j for j, f in enumerate(not_idx)}
    for c in range(NCORES):
        dev = results[c]["out"]  # [32, 128, 512]
        rows = slice(c * BS, (c + 1) * BS)
        out[rows, :NAND] = dev[:NAND].transpose(1, 0, 2)
        out[rows, NAND : NAND + NOR] = dev[NAND:NPAIR].transpose(1, 0, 2)
        for pos, f in enumerate(leftover_idx.tolist()):
            col = NAND + NOR + pos
            if f in not_slot:
                out[rows, col] = dev[NPAIR + not_slot[f]]
            else:
                out[rows, col] = fv[rows, f]
    return out



# revision 10
# speedup vs baseline: 1.3569x; 1.3569x over previous
"""Trainium2 Bass kernel for nn_LogicalOperatorFusion.

Semantics (matches the jax reference):
  fv = field_vectors                                  [B=1024, NF=64, H=512] f32
  fv[:, not_idx] = tanh(fv[:, not_idx] @ not_W + not_b)
  cat_and = concat(fv[:, and_pairs[:,0]], fv[:, and_pairs[:,1]], -1)   [B,16,1024]
  fused_and = relu(cat_and @ and_W1 + and_b1) @ and_W2 + and_b2        [B,16,512]
  cat_or  = ... same with or_pairs (8 pairs) ...                       [B,8,512]
  out = concat([fused_and, fused_or, fv[:, leftover_idx]], axis=1)     [B,40,512]

Strategy (v3 — fully transposed dataflow):
  - Data parallel: batch sharded 8 ways (128 rows/core), weights replicated.
  - Everything is computed transposed: out^T[n, m] so the weight chunks are
    the PE stationary operand and the (host pre-transposed) activations are
    the moving operand.  Layer 1 then produces h^T directly, which is
    exactly the moving operand layer 2 needs — zero on-chip transposes and
    zero DVE work.
  - Biases become per-partition scalars in the transposed layout, so ACT
    applies them for free in the same pass as the relu/tanh/copy from PSUM.
  - Pairs are processed in groups (moving free dim = width*128); the first
    two groups are narrow (2 pairs) so the first input DMAs are small and
    real compute starts ~3.7us in.  Per group: L1 = 32 matmuls into 4 psum
    banks (c-outer/k-inner), 4 relu ACTs -> h^T bf16, L2 = 16 matmuls into
    the other 4 banks (k-outer/c-inner for ACT pipelining), 4 bias ACTs ->
    bf16 out tile, output DMA.
  - Input DMAs ride the SP HWDGE queue in need-order; output DMAs ride the
    gpsimd SWDGE queue so they never head-of-line-block inputs.  The final
    group's outputs go back on the (by then idle) SP HWDGE queue, split in
    two, to minimize the drain tail.
  - The PE p-state ramp + input DMA lead-in are covered by warm-up matmuls
    that accumulate exact zeros (memset tile) into group 0's psum banks
    (same free-size region, so start/stop accumulation stays consistent).
"""

import sys

for p in ("/opt/trn_rl_repo",):
    if p not in sys.path:
        sys.path.insert(0, p)

import numpy as np
import ml_dtypes

BF16_NP = ml_dtypes.bfloat16

import concourse.bacc as bacc
import concourse.mybir as mybir
from concourse.bass_utils import run_bass_kernel_spmd
from concourse.tile import TileContext

B, NF, H = 1024, 64, 512
NCORES = 8
BS = B // NCORES  # 128 rows per core
NAND, NOR, NNOT = 16, 8, 8
# pair-group widths: groups must not straddle the and/or boundary
AND_W = [2, 2, 4, 4, 4]
OR_W = [4, 4]
PAIR_W = AND_W + OR_W
NOT_W = [4, 2, 2]
NG_PAIR = len(PAIR_W)
KC1 = (2 * H) // 128  # 8 k-chunks for layer 1
KC2 = H // 128  # 4 k-chunks for layer 2 / not
F32 = mybir.dt.float32
BF16 = mybir.dt.bfloat16
ACTF = mybir.ActivationFunctionType

# columns of the [128, 20] bias tile: and_b1 | or_b1 | and_b2 | or_b2 | not_b
B1_AND, B1_OR, B2_AND, B2_OR, B_NOT = 0, 4, 8, 12, 16

WARMUP_MM = 14  # zero-accumulating warmups covering DMA lead-in + pstate ramp

# flat column offsets in the mv / out dram tensors
MV_OFF = np.cumsum([0] + [KC1 * w * 128 for w in PAIR_W]).tolist()
MVN_OFF = np.cumsum([0] + [KC2 * w * 128 for w in NOT_W]).tolist()
OUT_OFF = np.cumsum([0] + [KC2 * w * 128 for w in PAIR_W + NOT_W]).tolist()

TRACE = False  # test.py sets this for profiled runs
LAST_RESULT = None  # BassKernelResults of the last run (for test.py)

_NC = None  # cached traced Bass program


def _build_nc():
    """Trace the per-core Bass program (identical on all 8 cores)."""
    nc = bacc.Bacc("TRN2", target_bir_lowering=False)

    mv_d = nc.dram_tensor("mv", [128, MV_OFF[-1]], BF16, kind="ExternalInput")
    mvn_d = nc.dram_tensor("mvn", [128, MVN_OFF[-1]], BF16, kind="ExternalInput")
    w1a_d = nc.dram_tensor("w1a", [128, KC2 * 1024], BF16, kind="ExternalInput")
    w1o_d = nc.dram_tensor("w1o", [128, KC2 * 1024], BF16, kind="ExternalInput")
    w2a_d = nc.dram_tensor("w2a", [128, KC2 * 512], BF16, kind="ExternalInput")
    w2o_d = nc.dram_tensor("w2o", [128, KC2 * 512], BF16, kind="ExternalInput")
    wn_d = nc.dram_tensor("wn", [128, KC2 * 512], BF16, kind="ExternalInput")
    bias_d = nc.dram_tensor("bias", [128, 20], F32, kind="ExternalInput")
    out_d = nc.dram_tensor("out", [128, OUT_OFF[-1]], BF16, kind="ExternalOutput")

    w0cols = AND_W[0] * 128  # first group's psum free size (warmup must match)

    with TileContext(nc) as tc:
        with (
            tc.tile_pool(name="consts", bufs=1) as consts,
            tc.tile_pool(name="wts", bufs=1) as wts,
            tc.tile_pool(name="mvp", bufs=2 * NG_PAIR) as mvp,
            tc.tile_pool(name="mvnp", bufs=len(NOT_W)) as mvnp,
            tc.tile_pool(name="htp", bufs=2) as htp,
            tc.tile_pool(name="osbp", bufs=3) as osbp,
            tc.tile_pool(name="ps1", bufs=1, space="PSUM") as ps1p,
            tc.tile_pool(name="ps2", bufs=1, space="PSUM") as ps2p,
        ):
            # ---- warmup: PE busy (and ramping) from t~0 on exact zeros ----
            dummy = consts.tile([128, w0cols], BF16, tag="dummy")
            nc.vector.memset(dummy, 0.0)
            warm_ps1 = [
                ps1p.tile([128, w0cols], F32, tag=f"c{c}", name=f"ps1w_{c}")
                for c in range(4)
            ]
            for i in range(WARMUP_MM):
                nc.tensor.matmul(
                    warm_ps1[i % 4],
                    dummy[:, 0:128],
                    dummy[:, :],
                    start=(i < 4),
                    stop=False,
                )

            # ---- tiles ----
            bias_sb = consts.tile([128, 20], F32, tag="bias")
            w1a_sb = [
                wts.tile([128, 1024], BF16, tag=f"w1a{c}", name=f"w1a_{c}")
                for c in range(4)
            ]
            w1o_sb = wts.tile([128, KC2 * 1024], BF16, tag="w1o")
            w2a_sb = [
                wts.tile([128, 512], BF16, tag=f"w2a{c}", name=f"w2a_{c}")
                for c in range(4)
            ]
            w2o_sb = wts.tile([128, KC2 * 512], BF16, tag="w2o")
            wn_sb = wts.tile([128, KC2 * 512], BF16, tag="wn")
            # per pair group: two moving tiles (k-chunks 0-3 / 4-7)
            mv_sb = [
                [
                    mvp.tile([128, 4 * w * 128], BF16, tag="mv", name=f"mv_{g}_{h}")
                    for h in range(2)
                ]
                for g, w in enumerate(PAIR_W)
            ]
            mvn_sb = [
                mvnp.tile([128, KC2 * w * 128], BF16, tag="mvn", name=f"mvn_{n}")
                for n, w in enumerate(NOT_W)
            ]

            def dma_mv(g):
                half = 4 * PAIR_W[g] * 128
                o = MV_OFF[g]
                nc.sync.dma_start(out=mv_sb[g][0], in_=mv_d[:, o : o + half])
                nc.sync.dma_start(out=mv_sb[g][1], in_=mv_d[:, o + half : o + 2 * half])

            # ---- input DMAs on the SP HWDGE queue, in arrival-vs-need order
            # (first-group chain: mv0a -> w1c0 -> mv0b gets the first real
            # matmul to ~3.3us; w2a must land before A0's L2 at ~7us) ----
            half0 = 4 * PAIR_W[0] * 128
            nc.sync.dma_start(out=mv_sb[0][0], in_=mv_d[:, 0:half0])
            nc.sync.dma_start(out=w1a_sb[0], in_=w1a_d[:, 0:1024])
            nc.sync.dma_start(out=mv_sb[0][1], in_=mv_d[:, half0 : 2 * half0])
            nc.sync.dma_start(out=w1a_sb[1], in_=w1a_d[:, 1024:2048])
            nc.sync.dma_start(out=bias_sb, in_=bias_d[:, :])
            nc.sync.dma_start(out=w1a_sb[2], in_=w1a_d[:, 2048:3072])
            nc.sync.dma_start(out=w1a_sb[3], in_=w1a_d[:, 3072:4096])
            nc.sync.dma_start(out=w2a_sb[0], in_=w2a_d[:, 0:512])
            nc.sync.dma_start(out=w2a_sb[1], in_=w2a_d[:, 512:1024])
            nc.sync.dma_start(out=mv_sb[1][0], in_=mv_d[:, MV_OFF[1] : MV_OFF[1] + 1024])
            nc.sync.dma_start(out=w2a_sb[2], in_=w2a_d[:, 1024:1536])
            nc.sync.dma_start(out=w2a_sb[3], in_=w2a_d[:, 1536:2048])
            nc.sync.dma_start(
                out=mv_sb[1][1], in_=mv_d[:, MV_OFF[1] + 1024 : MV_OFF[2]]
            )
            dma_mv(2)
            dma_mv(3)
            nc.sync.dma_start(out=wn_sb, in_=wn_d[:, :])
            nc.sync.dma_start(
                out=mvn_sb[0], in_=mvn_d[:, MVN_OFF[0] : MVN_OFF[1]]
            )
            dma_mv(4)
            nc.sync.dma_start(out=w1o_sb, in_=w1o_d[:, :])
            nc.sync.dma_start(out=w2o_sb, in_=w2o_d[:, :])
            dma_mv(5)
            dma_mv(6)
            nc.sync.dma_start(
                out=mvn_sb[1], in_=mvn_d[:, MVN_OFF[1] : MVN_OFF[2]]
            )
            nc.sync.dma_start(
                out=mvn_sb[2], in_=mvn_d[:, MVN_OFF[2] : MVN_OFF[3]]
            )

            # ---- compute ----
            def emit_pair_group(g, is_and, out_slot, ps1_override=None, l2_c_outer=False):
                w = PAIR_W[g]
                mcols = w * 128
                if is_and:
                    w1ap = lambda c, k: w1a_sb[c][:, k * 128 : (k + 1) * 128]
                    w2ap = lambda c, k: w2a_sb[c][:, k * 128 : (k + 1) * 128]
                    b1o, b2o = B1_AND, B2_AND
                else:
                    w1ap = lambda c, k: w1o_sb[
                        :, c * 1024 + k * 128 : c * 1024 + (k + 1) * 128
                    ]
                    w2ap = lambda c, k: w2o_sb[
                        :, c * 512 + k * 128 : c * 512 + (k + 1) * 128
                    ]
                    b1o, b2o = B1_OR, B2_OR
                mva, mvb = mv_sb[g]

                if ps1_override is None:
                    ps1 = [
                        ps1p.tile([128, mcols], F32, tag=f"c{c}", name=f"ps1g_{c}")
                        for c in range(4)
                    ]
                    start0 = True
                else:
                    ps1 = ps1_override  # warmups already opened these banks
                    start0 = False
                # L1: c-outer, k-inner (early psum stops -> early h^T ACTs)
                for c in range(4):
                    for k in range(KC1):
                        mv_t = mva if k < 4 else mvb
                        nc.tensor.matmul(
                            ps1[c],
                            w1ap(c, k),
                            mv_t[:, (k % 4) * mcols : (k % 4 + 1) * mcols],
                            start=(k == 0 and start0),
                            stop=(k == KC1 - 1),
                        )
                ht = [
                    htp.tile([128, mcols], BF16, tag=f"ht{c}", name=f"ht_{c}")
                    for c in range(4)
                ]
                for c in range(4):
                    nc.scalar.activation(
                        ht[c], ps1[c], ACTF.Relu, bias=bias_sb[:, b1o + c : b1o + c + 1]
                    )
                # L2: k-outer, c-inner (each ht tile feeds 4 back-to-back matmuls)
                ps2 = [
                    ps2p.tile([128, mcols], F32, tag=f"o{c}", name=f"ps2_{c}")
                    for c in range(4)
                ]
                # c2-outer: bank c2 stops 25% into L2 so the DVE out-stage
                # and the output DMA drain while L2 still runs (needed for the
                # first group -- w2 chunks still landing -- and the last pair
                # group, whose drain must hide behind the not-groups).  Mid
                # groups use k2-outer, which never waits on the late ht3.
                l2_iter = (
                    [(c2, k2) for c2 in range(4) for k2 in range(KC2)]
                    if l2_c_outer
                    else [(c2, k2) for k2 in range(KC2) for c2 in range(4)]
                )
                for c2, k2 in l2_iter:
                    nc.tensor.matmul(
                        ps2[c2],
                        w2ap(c2, k2),
                        ht[k2][:, :],
                        start=(k2 == 0),
                        stop=(k2 == KC2 - 1),
                    )
                osb = osbp.tile([128, 4 * mcols], BF16, tag="osb")
                for c2 in range(4):
                    # bias-add + bf16 downcast on the otherwise-idle DVE, so
                    # ACT only ever runs the relu/tanh chains
                    nc.vector.tensor_scalar_add(
                        osb[:, c2 * mcols : (c2 + 1) * mcols],
                        ps2[c2],
                        bias_sb[:, b2o + c2 : b2o + c2 + 1],
                    )
                o = OUT_OFF[out_slot]
                nc.sync.dma_start(out=out_d[:, o : o + 4 * mcols], in_=osb)

            def emit_not_group(n, out_slot, last=False):
                w = NOT_W[n]
                mcols = w * 128
                mv_t = mvn_sb[n]
                ps1 = [
                    ps1p.tile([128, mcols], F32, tag=f"c{c}", name=f"ps1n_{c}")
                    for c in range(4)
                ]
                # c-outer, k-inner: bank c stops after 4 matmuls -> tanh drains early
                for c in range(4):
                    for k in range(KC2):
                        nc.tensor.matmul(
                            ps1[c],
                            wn_sb[:, c * 512 + k * 128 : c * 512 + (k + 1) * 128],
                            mv_t[:, k * mcols : (k + 1) * mcols],
                            start=(k == 0),
                            stop=(k == KC2 - 1),
                        )
                osb = osbp.tile([128, 4 * mcols], BF16, tag="osb")
                o = OUT_OFF[out_slot]
                for c in range(4):
                    nc.scalar.activation(
                        osb[:, c * mcols : (c + 1) * mcols],
                        ps1[c],
                        ACTF.Tanh,
                        bias=bias_sb[:, B_NOT + c : B_NOT + c + 1],
                    )
                    if last and c == 1:
                        nc.sync.dma_start(
                            out=out_d[:, o : o + 2 * mcols], in_=osb[:, 0 : 2 * mcols]
                        )
                if last:
                    nc.sync.dma_start(
                        out=out_d[:, o + 2 * mcols : o + 4 * mcols],
                        in_=osb[:, 2 * mcols : 4 * mcols],
                    )
                else:
                    nc.sync.dma_start(out=out_d[:, o : o + 4 * mcols], in_=osb)

            # schedule: and groups, N0, or groups, N1 (not last: single-stage
            # c-outer shape drains psum->ACT->DMA with minimal tail)
            emit_pair_group(0, True, 0, ps1_override=warm_ps1, l2_c_outer=True)
            emit_pair_group(1, True, 1)
            emit_pair_group(2, True, 2)
            emit_pair_group(3, True, 3)
            emit_not_group(0, NG_PAIR)
            emit_pair_group(4, True, 4)
            emit_pair_group(5, False, 5)
            emit_pair_group(6, False, 6, l2_c_outer=True)
            emit_not_group(1, NG_PAIR + 1)
            emit_not_group(2, NG_PAIR + 2, last=True)

    nc.compile()
    return nc


def _get_nc():
    global _NC
    if _NC is None:
        _NC = _build_nc()
    return _NC


def _w1_tiles(W):  # [2H, H] -> [128, 4096]; [p, c*1024+k*128+q] = W[k*128+p, c*128+q]
    return np.ascontiguousarray(
        W.reshape(KC1, 128, 4, 128)
        .transpose(1, 2, 0, 3)
        .reshape(128, KC1 * 512)
        .astype(BF16_NP)
    )


def _w2_tiles(W):  # [H, H] -> [128, 2048]; [p, c*512+k*128+q] = W[k*128+p, c*128+q]
    return np.ascontiguousarray(
        W.reshape(KC2, 128, 4, 128)
        .transpose(1, 2, 0, 3)
        .reshape(128, KC2 * 512)
        .astype(BF16_NP)
    )


def kernel(
    field_vectors,
    and_W1,
    and_b1,
    and_W2,
    and_b2,
    or_W1,
    or_b1,
    or_W2,
    or_b2,
    not_W,
    not_b,
    not_idx,
    and_pairs,
    or_pairs,
    leftover_idx,
):
    global LAST_RESULT
    fv = np.asarray(field_vectors, dtype=np.float32)
    and_W1 = np.asarray(and_W1, dtype=np.float32)
    and_W2 = np.asarray(and_W2, dtype=np.float32)
    or_W1 = np.asarray(or_W1, dtype=np.float32)
    or_W2 = np.asarray(or_W2, dtype=np.float32)
    not_W = np.asarray(not_W, dtype=np.float32)
    not_idx = np.asarray(not_idx).astype(np.int64).ravel()
    and_pairs = np.asarray(and_pairs).astype(np.int64).reshape(NAND, 2)
    or_pairs = np.asarray(or_pairs).astype(np.int64).reshape(NOR, 2)
    leftover_idx = np.asarray(leftover_idx).astype(np.int64).ravel()

    not_set = set(not_idx.tolist())
    pair_fields = np.concatenate([and_pairs, or_pairs])  # [24, 2]
    # The reference applies the not-transform before gathering pairs; with the
    # disjoint index structure used by this problem, pair fields are untouched.
    assert not (set(pair_fields.ravel().tolist()) & not_set), (
        "pair fields overlapping not_idx not supported"
    )

    # [128, 20] f32 bias tile: cols = and_b1 | or_b1 | and_b2 | or_b2 | not_b
    bias_tile = np.concatenate(
        [
            np.asarray(v, np.float32).reshape(4, 128).T
            for v in (and_b1, or_b1, and_b2, or_b2, not_b)
        ],
        axis=1,
    )

    shared = {
        "w1a": _w1_tiles(and_W1),
        "w1o": _w1_tiles(or_W1),
        "w2a": _w2_tiles(and_W2),
        "w2o": _w2_tiles(or_W2),
        "wn": _w2_tiles(not_W),
        "bias": np.ascontiguousarray(bias_tile),
    }

    # group -> (first pair index, width)
    pair_groups = []
    p0 = 0
    for w in PAIR_W:
        pair_groups.append((p0, w))
        p0 += w
    not_groups = []
    n0 = 0
    for w in NOT_W:
        not_groups.append((n0, w))
        n0 += w

    in_maps = []
    for core in range(NCORES):
        fv_c = fv[core * BS : (core + 1) * BS]  # [128, 64, 512]
        mv = np.empty((128, MV_OFF[-1]), dtype=BF16_NP)
        for g, (ps, w) in enumerate(pair_groups):
            # [p, k*(w*128) + j*128 + m] = fv_c[m, field(j,s), (k%4)*128+p]
            F = fv_c[:, pair_fields[ps : ps + w], :]  # [m, j, s, h]
            mv[:, MV_OFF[g] : MV_OFF[g + 1]] = (
                F.reshape(BS, w, 2, KC2, 128)
                .transpose(4, 2, 3, 1, 0)  # p s c j m
                .reshape(128, KC1 * w * 128)
            )
        mvn = np.empty((128, MVN_OFF[-1]), dtype=BF16_NP)
        for n, (ns, w) in enumerate(not_groups):
            N = fv_c[:, not_idx[ns : ns + w], :]  # [m, j, h]
            mvn[:, MVN_OFF[n] : MVN_OFF[n + 1]] = (
                N.reshape(BS, w, KC2, 128)
                .transpose(3, 2, 1, 0)  # p k j m
                .reshape(128, KC2 * w * 128)
            )
        in_maps.append({"mv": mv, "mvn": mvn, **shared})

    nc = _get_nc()
    res = run_bass_kernel_spmd(nc, in_maps, core_ids=list(range(NCORES)), trace=TRACE)
    LAST_RESULT = res
    results = res.results if hasattr(res, "results") else res

    out = np.empty((B, NAND + NOR + len(leftover_idx), H), dtype=np.float32)
    not_slot = {int(f): j for j, f in enumerate(not_idx)}
    for core in range(NCORES):
        dev = np.asarray(results[core]["out"], dtype=np.float32)  # [128, OUT_OFF[-1]]
        rows = slice(core * BS, (core + 1) * BS)

        def unpack(slot, w):  # -> [w, 128, 512]; Y[j][m, c*128+p]
            o = OUT_OFF[slot]
            R = dev[:, o : o + 4 * w * 128]
            return (
                R.reshape(128, 4, w, 128).transpose(2, 3, 1, 0).reshape(w, BS, H)
            )

        for g, (ps, w) in enumerate(pair_groups):
            out[rows, ps : ps + w] = unpack(g, w).transpose(1, 0, 2)
        nres = [unpack(NG_PAIR + n, w) for n, (ns, w) in enumerate(not_groups)]
        nmap = [(n, i) for n, (ns, w) in enumerate(not_groups) for i in range(w)]
        for pos, f in enumerate(leftover_idx.tolist()):
            col = NAND + NOR + pos
            if f in not_slot:
                n, i = nmap[not_slot[f]]
                out[rows, col] = nres[n][i]
            else:
                out[rows, col] = fv[rows, f]
    return out
